# revision 1
# baseline (speedup 1.0000x reference)
"""LSTM sequence classifier on 8 Trainium2 NeuronCores.

Data-parallel over batch: each core gets ~1/8 of the 4096 sequences.
Per core: dma_gather (transpose mode) pulls token embeddings from the
bf16 table in HBM directly into feature-major SBUF layout; a fully
unrolled 22-step LSTM runs as bf16 matmuls (fp32 PSUM accumulate) with
ACT sigmoid/tanh drains and DVE cell updates. Batches are sorted by
sequence length (descending) and dealt so all cores share an identical
length multiset; per-step work shrinks to the still-active prefix and
final hidden states are captured by column-range copies.
"""
import sys

sys.path.insert(0, "/opt/trn_rl_repo")

import numpy as np
import ml_dtypes

import concourse.bass as bass
import concourse.tile as tile
from concourse import bacc, mybir
from concourse.bass_utils import run_bass_kernel_spmd

V, E, H, T, B = 30000, 300, 300, 22, 4096
NCORES = 8
EP = 384          # padded embedding row (elements); 768 B in bf16
GP = 384          # padded rows per gate (3 K-tiles of 128)
MW = 4 * GP       # 1536 padded gate rows total
NMT = MW // 128   # 12 M-tiles
KT = 3            # K-tiles per operand (300 -> 128,128,44)
CS = 1536         # gather chunk length (multiple of 128)
F32 = mybir.dt.float32
BF16 = mybir.dt.bfloat16
I16 = mybir.dt.int16
AF = mybir.ActivationFunctionType

_patched = False


def _patch_tile_drain():
    """walrus CTRL (Drain) supports fewer sem waits than Tile attaches at
    the kernel tail; spread them across single-wait SP NOPs instead."""
    global _patched
    if _patched:
        return
    _patched = True
    import concourse.tile as tile_mod
    from concourse.vector_clock import ScopedClock

    def _drain_and_barrier(self, tick_clock, wait_clock):
        nc = self.nc
        probe = nc.sync.nop(nofuse=True)
        wait_clock.add_sem_waits(
            probe.ins, ScopedClock({None: tick_clock.global_clock}))
        si = probe.ins.sync_info
        waits = list(si.on_wait) if si is not None else []
        upds = list(si.on_update) if si is not None else []
        probe.ins.sync_info = mybir.SyncInfo(on_wait=waits[:1], on_update=upds)
        for w in waits[1:]:
            n2 = nc.sync.nop(nofuse=True)
            n2.ins.sync_info = mybir.SyncInfo(on_wait=[w], on_update=[])
        nc.sync.drain()
        nc.all_engine_barrier()
        popped = nc._tile_sem_poison_stack.pop()
        assert popped is self._sem_poison
        nc.clear_and_free_semaphores(list(self.sems.allocated().values()))
        nc.all_engine_barrier()

    tile_mod.TileContext._drain_and_barrier = _drain_and_barrier


def _schedule(cap_len):
    """Deal batches to cores so every core has the same length multiset.

    Returns orders ([NCORES][Q] of global index or -1 for dummy) and the
    per-step active counts n_t (identical across cores).
    """
    q = np.zeros(T + 1, np.int64)  # q[l] = per-core count of length l
    orders = [[] for _ in range(NCORES)]
    for l in range(T, 0, -1):
        idxs = np.nonzero(cap_len == l)[0]
        k = len(idxs)
        ql = -(-k // NCORES)  # ceil
        q[l] = ql
        for c in range(NCORES):
            part = idxs[c::NCORES]
            orders[c].extend(int(x) for x in part)
            orders[c].extend([-1] * (ql - len(part)))
    n_t = [int(q[t + 1:].sum()) for t in range(T)]  # active at step t
    return orders, n_t


def _build_program(n_t, Q, NTOKP, chunks, offs):
    nc = bacc.Bacc("TRN2", target_bir_lowering=False, debug=False)
    emb_d = nc.dram_tensor("emb", [V, EP], BF16, kind="ExternalInput")
    idx_d = nc.dram_tensor("idx", [128, NTOKP // 16], I16, kind="ExternalInput")
    wx_d = nc.dram_tensor("wx", [KT, 128, MW], BF16, kind="ExternalInput")
    wh_d = nc.dram_tensor("wh", [KT, 128, MW], BF16, kind="ExternalInput")
    b_d = nc.dram_tensor("b", [128, NMT], F32, kind="ExternalInput")
    vt_d = nc.dram_tensor("vt", [KT, 128, 2], F32, kind="ExternalInput")
    g_d = nc.dram_tensor("g", [2, 1], F32, kind="ExternalInput")
    bc_d = nc.dram_tensor("bc", [2, 1], F32, kind="ExternalInput")
    eye_d = nc.dram_tensor("eye", [2, 2], F32, kind="ExternalInput")
    out_d = nc.dram_tensor("out", [2, Q], F32, kind="ExternalOutput")

    QR = -(-Q // 8) * 8
    gatebuf_names = ["ib", "fb", "gb", "ob"]
    gatefunc = [AF.Sigmoid, AF.Sigmoid, AF.Tanh, AF.Sigmoid]

    with tile.TileContext(nc) as tc:
        with (
            tc.tile_pool(name="const", bufs=1) as cpool,
            tc.tile_pool(name="xt", bufs=1) as xpool,
            tc.tile_pool(name="state", bufs=1) as spool,
            tc.tile_pool(name="gates", bufs=1) as gpool,
            tc.tile_pool(name="ps", bufs=6, space="PSUM") as pspool,
            tc.tile_pool(name="psh", bufs=1, space="PSUM") as hpool,
        ):
            wx_sb = cpool.tile([128, KT, MW], BF16, tag="wx")
            wh_sb = cpool.tile([128, KT, MW], BF16, tag="wh")
            for k in range(KT):
                nc.sync.dma_start(out=wx_sb[:, k, :], in_=wx_d[k])
                nc.sync.dma_start(out=wh_sb[:, k, :], in_=wh_d[k])
            b_sb = cpool.tile([128, NMT], F32, tag="b")
            nc.sync.dma_start(out=b_sb[:], in_=b_d[:])
            vt_sb = cpool.tile([128, KT, 2], F32, tag="vt")
            for k in range(KT):
                nc.sync.dma_start(out=vt_sb[:, k, :], in_=vt_d[k])
            g_sb = cpool.tile([2, 1], F32, tag="g")
            nc.sync.dma_start(out=g_sb[:], in_=g_d[:])
            bc_sb = cpool.tile([2, 1], F32, tag="bc")
            nc.sync.dma_start(out=bc_sb[:], in_=bc_d[:])
            eye_sb = cpool.tile([2, 2], F32, tag="eye")
            nc.sync.dma_start(out=eye_sb[:], in_=eye_d[:])
            idx_sb = cpool.tile([128, NTOKP // 16], I16, tag="idx")
            nc.sync.dma_start(out=idx_sb[:], in_=idx_d[:])

            # head scale s = g / ||v|| (independent of the recurrence)
            ssq_ps = hpool.tile([2, 2], F32, tag="ph2")
            for k in range(KT):
                nc.tensor.matmul(ssq_ps[:], vt_sb[:, k, :], vt_sb[:, k, :],
                                 start=(k == 0), stop=(k == KT - 1))
            masked = spool.tile([2, 2], F32, tag="masked")
            nc.vector.tensor_mul(masked[:], ssq_ps[:], eye_sb[:])
            ssq = spool.tile([2, 1], F32, tag="ssq")
            nc.vector.reduce_sum(ssq[:], masked[:], axis=mybir.AxisListType.X)
            rinv = spool.tile([2, 1], F32, tag="rinv")
            nc.vector.reciprocal(rinv[:], ssq[:])
            rsq = spool.tile([2, 1], F32, tag="rsq")
            nc.scalar.activation(rsq[:], rinv[:], AF.Sqrt)
            s_sb = spool.tile([2, 1], F32, tag="s")
            nc.vector.tensor_mul(s_sb[:], rsq[:], g_sb[:])

            # gather chunks (feature-major bf16: xt[q, c, i] = emb[tok_i, 128c+q])
            xts = []
            for ci, (s0, s1) in enumerate(chunks):
                xt = xpool.tile([128, KT, s1 - s0], BF16, tag=f"xt{ci}")
                nc.gpsimd.dma_gather(
                    out_ap=xt[:], in_ap=emb_d[:],
                    idxs_ap=idx_sb[:, s0 // 16:s1 // 16],
                    num_idxs=s1 - s0, num_idxs_reg=s1 - s0,
                    elem_size=EP, transpose=True, single_packet=False)
                xts.append(xt)

            hT = spool.tile([128, KT, QR], BF16, tag="hT")
            cT = spool.tile([128, KT, QR], F32, tag="cT")
            tanh_c = spool.tile([128, KT, QR], F32, tag="tanh_c")
            tmp = spool.tile([128, KT, QR], F32, tag="tmp")
            lastT = spool.tile([128, KT, QR], F32, tag="lastT")
            gbufs = []
            for nm in gatebuf_names:
                gt = gpool.tile([128, KT, QR], F32, tag=nm, name=nm)
                gbufs.append(gt)

            for t in range(T):
                n = n_t[t]
                if n == 0:
                    continue
                off = offs[t]
                # segments: split at 512 cols and at gather-chunk crossings
                segs = []
                col = 0
                while col < n:
                    p = off + col
                    ci = next(i for i, (s0, s1) in enumerate(chunks)
                              if s0 <= p < s1)
                    end = min(n, chunks[ci][1] - off, col + 512)
                    segs.append((col, end, ci, p - chunks[ci][0]))
                    col = end
                for m in range(NMT):
                    g = m // KT
                    sub = m % KT
                    for (lo, hi, ci, a) in segs:
                        w = hi - lo
                        ps = pspool.tile([128, 512], F32, tag="ps")
                        nmm = 2 * KT if t > 0 else KT
                        i_mm = 0
                        for k in range(KT):
                            nc.tensor.matmul(
                                ps[:, :w],
                                wx_sb[:, k, m * 128:(m + 1) * 128],
                                xts[ci][:, k, a:a + w],
                                start=(i_mm == 0), stop=(i_mm == nmm - 1))
                            i_mm += 1
                        if t > 0:
                            for k in range(KT):
                                nc.tensor.matmul(
                                    ps[:, :w],
                                    wh_sb[:, k, m * 128:(m + 1) * 128],
                                    hT[:, k, lo:hi],
                                    start=False, stop=(i_mm == nmm - 1))
                                i_mm += 1
                        nc.scalar.activation(
                            gbufs[g][:, sub, lo:hi], ps[:, :w], gatefunc[g],
                            bias=b_sb[:, m:m + 1], scale=1.0)
                ib, fb, gb, ob = gbufs
                if t == 0:
                    nc.vector.tensor_mul(cT[:, :, :n], ib[:, :, :n], gb[:, :, :n])
                else:
                    nc.vector.tensor_mul(tmp[:, :, :n], ib[:, :, :n], gb[:, :, :n])
                    nc.vector.tensor_mul(cT[:, :, :n], fb[:, :, :n], cT[:, :, :n])
                    nc.vector.tensor_add(cT[:, :, :n], cT[:, :, :n], tmp[:, :, :n])
                nc.scalar.activation(tanh_c[:, :, :n], cT[:, :, :n], AF.Tanh)
                cap_lo = n_t[t + 1] if t < T - 1 else 0
                if cap_lo < n:
                    nc.vector.tensor_mul(lastT[:, :, cap_lo:n],
                                         ob[:, :, cap_lo:n],
                                         tanh_c[:, :, cap_lo:n])
                if t < T - 1 and cap_lo > 0:
                    nc.vector.tensor_mul(hT[:, :, :cap_lo], ob[:, :, :cap_lo],
                                         tanh_c[:, :, :cap_lo])

            # head: logits^T = s * (v @ last^T) + b_cls
            out_sb = spool.tile([2, QR], F32, tag="out_sb")
            col = 0
            while col < Q:
                w = min(512, Q - col)
                ph = hpool.tile([2, 512], F32, tag="ph")
                for k in range(KT):
                    nc.tensor.matmul(ph[:, :w], vt_sb[:, k, :],
                                     lastT[:, k, col:col + w],
                                     start=(k == 0), stop=(k == KT - 1))
                nc.scalar.activation(out_sb[:, col:col + w], ph[:, :w],
                                     AF.Identity, bias=bc_sb[:, 0:1],
                                     scale=s_sb[:, 0:1])
                col += w
            nc.sync.dma_start(out=out_d[:], in_=out_sb[:, :Q])

    nc.compile()
    return nc


def _prep_and_run(inputs, trace=False):
    _patch_tile_drain()
    cap = np.asarray(inputs["cap"]).astype(np.int64)
    cap_len = np.asarray(inputs["cap_len"]).astype(np.int64)
    embed = np.asarray(inputs["embed"], np.float32)
    W_ih = np.asarray(inputs["W_ih"], np.float32)
    W_hh = np.asarray(inputs["W_hh"], np.float32)
    b_ih = np.asarray(inputs["b_ih"], np.float32)
    b_hh = np.asarray(inputs["b_hh"], np.float32)
    v_wn = np.asarray(inputs["v_wn"], np.float32)
    g_wn = np.asarray(inputs["g_wn"], np.float32)
    b_cls = np.asarray(inputs["b_cls"], np.float32)

    orders, n_t = _schedule(cap_len)
    Q = n_t[0]
    offs = np.concatenate([[0], np.cumsum(n_t)]).astype(np.int64)
    NTOK = int(offs[-1])
    NTOKP = -(-NTOK // 128) * 128

    # per-core token streams, packed for dma_gather (idx i -> [i%16, i//16])
    idx_maps = []
    for c in range(NCORES):
        order = np.asarray(orders[c], np.int64)
        toks = np.zeros(NTOKP, np.int16)
        for t in range(T):
            n = n_t[t]
            sel = order[:n]
            tk = np.where(sel >= 0, cap[np.clip(sel, 0, None), t], 0)
            toks[offs[t]:offs[t] + n] = tk.astype(np.int16)
        packed = np.tile(toks.reshape(NTOKP // 16, 16).T, (8, 1)).copy()
        idx_maps.append(packed)

    # graded chunks: small first chunks so early steps start sooner
    # (all gathers serialize on SWDGE queue 0)
    sizes = [640, 512, 1024]
    chunks = []
    s = 0
    while s < NTOKP:
        cl = sizes.pop(0) if sizes else CS
        chunks.append((s, min(s + cl, NTOKP)))
        s += cl

    # weights: lhsT layouts
    emb_pad = np.zeros((V, EP), ml_dtypes.bfloat16)
    emb_pad[:, :E] = embed.astype(ml_dtypes.bfloat16)

    def pack_w(Wmat, kdim):
        Wp = np.zeros((MW, EP), np.float32)
        for g in range(4):
            Wp[GP * g:GP * g + H, :kdim] = Wmat[H * g:H * g + H, :]
        return np.ascontiguousarray(
            Wp.T.reshape(KT, 128, MW)).astype(ml_dtypes.bfloat16)

    wx_np = pack_w(W_ih, E)
    wh_np = pack_w(W_hh, H)
    b_pad = np.zeros(MW, np.float32)
    for g in range(4):
        b_pad[GP * g:GP * g + H] = (b_ih + b_hh)[H * g:H * g + H]
    b_np = np.ascontiguousarray(b_pad.reshape(NMT, 128).T)
    v_pad = np.zeros((2, EP), np.float32)
    v_pad[:, :H] = v_wn
    vt_np = np.ascontiguousarray(v_pad.T.reshape(KT, 128, 2))
    g_np = np.ascontiguousarray(g_wn.reshape(2, 1))
    bc_np = np.ascontiguousarray(b_cls.reshape(2, 1))
    eye_np = np.eye(2, dtype=np.float32)

    nc = _build_program(n_t, Q, NTOKP, chunks, offs)

    in_maps = []
    for c in range(NCORES):
        in_maps.append({
            "emb": emb_pad, "idx": idx_maps[c], "wx": wx_np, "wh": wh_np,
            "b": b_np, "vt": vt_np, "g": g_np, "bc": bc_np, "eye": eye_np,
        })
    res = run_bass_kernel_spmd(nc, in_maps, list(range(NCORES)), trace=trace)

    out = np.zeros((B, 2), np.float32)
    for c in range(NCORES):
        logitsT = res.results[c]["out"]  # [2, Q]
        order = orders[c]
        for pos, gi in enumerate(order):
            if gi >= 0:
                out[gi] = logitsT[:, pos]
    return out, res


def kernel(**inputs):
    out, _ = _prep_and_run(inputs, trace=False)
    return out

